# revision 10
# baseline (speedup 1.0000x reference)
"""Trainium2 Bass kernel for nn_NlsqCond (ConvFlow NLSQ coupling layer).

Strategy: pure data parallel over batch B=256 -> 32 samples per core on 8
NeuronCores. Convs are computed as 3 shifted matmuls accumulating in PSUM
over a gap-column activation layout (34 columns per sample, zero guard
columns), so the k=3/pad=1 conv needs no boundary special-casing. Weights
are transposed host-side into lhsT layout and cast to bf16 (fp32 PSUM
accumulation); measured end-to-end error vs fp32 reference is ~1e-4.
The final projection is computed transposed ([cols, 10]) so the NLSQ
elementwise tail runs with full 128-partition parallelism; the per-sample
logdet partition-reduction is done with a small mask matmul.
"""

import math

import numpy as np

B, T, D, H, COND = 256, 64, 2, 512, 8
T2 = T // 2                      # 32
NCORES = 8
NB = B // NCORES                 # 32 samples per core
ST = T2 + 2                      # 34: per-sample column stride (zero gaps)
WCOLS = NB * ST                  # 1088
NG = 2                           # PSUM column groups per matmul set
GS = NB // NG                    # 16 samples per group
NCHUNK = NB * T2 // 128          # 8 column chunks of 128 for final proj
LOG_A = math.log(8.0 * math.sqrt(3.0) / 9.0 - 0.05)

_CACHE = {}


def _build_program():
    import concourse.bacc as bacc
    import concourse.mybir as mybir
    import concourse.tile as tile

    f32 = mybir.dt.float32
    bf16 = mybir.dt.bfloat16
    AF = mybir.ActivationFunctionType
    Alu = mybir.AluOpType

    nc = bacc.Bacc("TRN2", target_bir_lowering=False, debug=False)

    # ---- DRAM I/O ----
    d_z1T = nc.dram_tensor("z1T", [2, NB * T2], bf16, kind="ExternalInput")
    d_z2r = nc.dram_tensor("z2r", [128, NCHUNK, 2], f32, kind="ExternalInput")
    d_condT = nc.dram_tensor("condT", [2 * COND, NB], bf16, kind="ExternalInput")
    d_we = nc.dram_tensor("we", [2, H], bf16, kind="ExternalInput")
    d_wc1 = nc.dram_tensor("wc1", [2 * COND, H], bf16, kind="ExternalInput")
    d_wc2 = nc.dram_tensor("wc2", [4, 128, H], bf16, kind="ExternalInput")
    d_wc0 = nc.dram_tensor("wc0", [8, 128, 3, H], bf16, kind="ExternalInput")
    d_wcv = nc.dram_tensor("wcv", [4, 4, 128, 3, H], bf16, kind="ExternalInput")
    d_wout = nc.dram_tensor("wout", [4, 128, 10], bf16, kind="ExternalInput")
    d_bias = nc.dram_tensor("biases", [128, 8, 4], f32, kind="ExternalInput")
    d_brep = nc.dram_tensor("brep", [128, NCHUNK, 10], f32, kind="ExternalInput")
    d_mask = nc.dram_tensor("mask", [128, 4], f32, kind="ExternalInput")
    d_z2n = nc.dram_tensor("z2n", [128, NCHUNK, 2], f32, kind="ExternalOutput")
    d_ld = nc.dram_tensor("ld", [NCHUNK, 4], f32, kind="ExternalOutput")

    with tile.TileContext(nc) as tc:
        with (
            tc.tile_pool(name="w", bufs=1) as wp,
            tc.tile_pool(name="act", bufs=1) as ap_,
            tc.tile_pool(name="ew", bufs=1) as ewp,
            tc.tile_pool(name="ps", bufs=8, space="PSUM") as pp,
        ):
            def mk(pool, shape, dtype, tag):
                return pool.tile(shape, dtype, tag=tag, name=tag)

            # ---- SBUF loads (small inputs first so PE can start early) ----
            z1T_sb = mk(ap_, [2, NB * T2], bf16, "z1T")
            nc.sync.dma_start(z1T_sb[:], d_z1T.ap())
            condT_sb = mk(ap_, [2 * COND, NB], bf16, "condT")
            nc.sync.dma_start(condT_sb[:], d_condT.ap())
            we_sb = mk(wp, [2, H], bf16, "we")
            nc.sync.dma_start(we_sb[:], d_we.ap())
            wc1_sb = mk(wp, [2 * COND, H], bf16, "wc1")
            nc.sync.dma_start(wc1_sb[:], d_wc1.ap())
            bias_sb = mk(wp, [128, 8, 4], f32, "bias")
            nc.sync.dma_start(bias_sb[:], d_bias.ap())
            wc2_sb = [mk(wp, [128, H], bf16, f"wc2_{i}") for i in range(4)]
            for i in range(4):
                nc.sync.dma_start(wc2_sb[i][:], d_wc2.ap()[i])
            wc0_sb = [mk(wp, [128, 3, H], bf16, f"wc0_{i}") for i in range(8)]
            for i in range(8):
                nc.sync.dma_start(wc0_sb[i][:], d_wc0.ap()[i])
            wcv_sb = [
                [mk(wp, [128, 3, H], bf16, f"wcv_{l}_{i}") for i in range(4)]
                for l in range(4)
            ]
            for l in range(4):
                for i in range(4):
                    nc.sync.dma_start(wcv_sb[l][i][:], d_wcv.ap()[l, i])
            wout_sb = [mk(wp, [128, 10], bf16, f"wout_{i}") for i in range(4)]
            for i in range(4):
                nc.sync.dma_start(wout_sb[i][:], d_wout.ap()[i])
            brep_sb = mk(wp, [128, NCHUNK, 10], f32, "brep")
            nc.sync.dma_start(brep_sb[:], d_brep.ap())
            mask_sb = mk(wp, [128, 4], f32, "mask")
            nc.sync.dma_start(mask_sb[:], d_mask.ap())
            z2r_sb = mk(ewp, [128, NCHUNK, 2], f32, "z2r")
            nc.sync.dma_start(z2r_sb[:], d_z2r.ap())

            def bias_ap(idx, mt):
                return bias_sb[:, idx, mt : mt + 1]

            # ---- activation ping-pong buffers (gap layout) ----
            # Only the gap guard columns need zeroing; valid columns are
            # always written before they are read.
            actA = [mk(ap_, [128, WCOLS], bf16, f"A{i}") for i in range(8)]
            actB = [mk(ap_, [128, WCOLS], bf16, f"B{i}") for i in range(4)]
            actC = [mk(ap_, [128, WCOLS], bf16, f"C{i}") for i in range(4)]
            for ti, t_ in enumerate(actA + actB + actC):
                v = t_[:].rearrange("p (s c) -> p s c", c=ST)
                eng = nc.vector if ti % 2 else nc.gpsimd
                eng.memset(v[:, :, 0:1], 0.0)
                eng.memset(v[:, :, ST - 1 : ST], 0.0)

            def valid(tl, g=None):
                v = tl[:].rearrange("p (s c) -> p s c", c=ST)
                if g is None:
                    return v[:, :, 1 : 1 + T2]
                return v[:, g * GS : (g + 1) * GS, 1 : 1 + T2]

            # ---- PE warm-up spin ----
            # The PE HAM clock gate starts at 1.2 GHz and only releases to
            # 2.4 GHz after ~3.4us of sustained activity. Matmul on scratch
            # zeros while the weight DMAs stream in, so the real conv stack
            # runs warm from its first instruction.
            wu_l = mk(ap_, [128, 128], bf16, "wu_l")
            wu_r = mk(ap_, [128, 512], bf16, "wu_r")
            nc.vector.memset(wu_l[:], 0.0)
            nc.vector.memset(wu_r[:], 0.0)
            ps_w = mk(pp, [128, 512], f32, "ps")
            for i in range(16):
                nc.tensor.matmul(
                    ps_w[:], wu_l[:], wu_r[:], start=(i == 0), stop=(i == 15)
                )

            # ---- embedding: h = z1 @ we.T + b_embed -> actA[0..3] ----
            for g in range(NG):
                for mt in range(4):
                    ps = mk(pp, [128, 512], f32, "ps")
                    nc.tensor.matmul(
                        ps[:],
                        we_sb[:, mt * 128 : (mt + 1) * 128],
                        z1T_sb[:, g * 512 : (g + 1) * 512],
                        start=True,
                        stop=True,
                    )
                    nc.scalar.activation(
                        valid(actA[mt], g),
                        ps[:].rearrange("p (s t) -> p s t", t=T2),
                        AF.Identity,
                        bias=bias_ap(0, mt),
                    )

            # ---- cond MLP: c2 = relu(W2 relu(W1 c + b1) + b2) ----
            c1_sb = [mk(ap_, [128, NB], bf16, f"c1_{i}") for i in range(4)]
            c2_sb = [mk(ap_, [128, NB], bf16, f"c2_{i}") for i in range(4)]
            for mt in range(4):
                ps = mk(pp, [128, 512], f32, "ps")
                nc.tensor.matmul(
                    ps[:, :NB],
                    wc1_sb[:, mt * 128 : (mt + 1) * 128],
                    condT_sb[:],
                    start=True,
                    stop=True,
                )
                nc.scalar.activation(
                    c1_sb[mt][:], ps[:, :NB], AF.Relu, bias=bias_ap(1, mt)
                )
            for mt in range(4):
                ps = mk(pp, [128, 512], f32, "ps")
                for kt in range(4):
                    nc.tensor.matmul(
                        ps[:, :NB],
                        wc2_sb[kt][:, mt * 128 : (mt + 1) * 128],
                        c1_sb[kt][:],
                        start=(kt == 0),
                        stop=(kt == 3),
                    )
                nc.scalar.activation(
                    c2_sb[mt][:], ps[:, :NB], AF.Relu, bias=bias_ap(2, mt)
                )
            # broadcast c2 over positions t -> actA[4..7]
            for mt in range(4):
                src = c2_sb[mt][:].unsqueeze(2).broadcast_to([128, NB, T2])
                nc.scalar.copy(valid(actA[4 + mt]), src)

            # ---- conv stack ----
            # Matmul moving operands must be single-free-dim, so each conv
            # matmul streams a contiguous window of the gap layout; outputs
            # at gap positions are garbage and simply never read back.
            WINDOWS = [(0, 15), (15, 15), (30, 2)]  # (sample base, n samples)
            srcs, ktn, wts = actA, 8, wc0_sb
            for L in range(5):
                dsts = actB if L % 2 == 0 else actC
                for mt in range(4):
                    pss = [mk(pp, [128, 512], f32, "ps") for _ in WINDOWS]
                    nacc = 3 * ktn
                    i = 0
                    # kt outer: matches weight-DMA arrival order, and for
                    # conv0 defers the cond-broadcast k-tiles (4..7) so the
                    # broadcast copies have more slack
                    for kt in range(ktn):
                        for k in range(3):
                            lhsT = wts[kt][:, k, mt * 128 : (mt + 1) * 128]
                            for wi, (sb, ns) in enumerate(WINDOWS):
                                n = ns * ST - 2
                                base = sb * ST + k
                                nc.tensor.matmul(
                                    pss[wi][:, :n],
                                    lhsT,
                                    srcs[kt][:, base : base + n],
                                    start=(i == 0),
                                    stop=(i == nacc - 1),
                                )
                            i += 1
                    for wi, (sb, ns) in enumerate(WINDOWS):
                        pv_ = pss[wi][:, : ns * ST].rearrange(
                            "p (s c) -> p s c", c=ST
                        )[:, :, 0:T2]
                        dv_ = dsts[mt][:].rearrange("p (s c) -> p s c", c=ST)[
                            :, sb : sb + ns, 1 : 1 + T2
                        ]
                        # split the PSUM->SBUF relu copies across ACT and DVE
                        if (mt * 3 + wi) % 2 == 0:
                            nc.scalar.activation(
                                dv_, pv_, AF.Relu, bias=bias_ap(3 + L, mt)
                            )
                        else:
                            nc.vector.tensor_scalar(
                                dv_,
                                pv_,
                                bias_ap(3 + L, mt),
                                0.0,
                                Alu.add,
                                Alu.max,
                            )
                srcs, ktn = dsts, 4
                if L < 4:
                    wts = wcv_sb[L]

            # ---- compact the final activation (drop gap columns) ----
            h5 = [mk(ap_, [128, NB * T2], bf16, f"h5_{i}") for i in range(4)]
            for kt in range(4):
                eng = nc.scalar if kt % 2 == 0 else nc.vector
                if kt % 2 == 0:
                    nc.scalar.copy(
                        h5[kt][:].rearrange("p (s t) -> p s t", t=T2),
                        valid(srcs[kt]),
                    )
                else:
                    nc.vector.tensor_copy(
                        h5[kt][:].rearrange("p (s t) -> p s t", t=T2),
                        valid(srcs[kt]),
                    )

            # ---- final projection (transposed): out[col, 10] ----
            pso = mk(pp, [128, 512], f32, "ps")
            for j in range(NCHUNK):
                for kt in range(4):
                    nc.tensor.matmul(
                        pso[:, j * 10 : (j + 1) * 10],
                        h5[kt][:, j * 128 : (j + 1) * 128],
                        wout_sb[kt][:],
                        start=(kt == 0),
                        stop=(kt == 3),
                    )

            # ---- NLSQ elementwise tail ----
            def ew(tag):
                return mk(ewp, [128, NCHUNK, 2], f32, tag)

            params = mk(ewp, [128, NCHUNK, 10], f32, "params")
            nc.vector.tensor_add(
                params[:],
                pso[:, : NCHUNK * 10].rearrange("p (j q) -> p j q", q=10),
                brep_sb[:],
            )
            pv = params[:].rearrange("p j (a q) -> p j a q", q=5)
            P0, P1, P2, P3, P4 = (pv[:, :, :, i] for i in range(5))

            loga_sb = mk(ewp, [128, 1], f32, "loga")
            nc.vector.memset(loga_sb[:], LOG_A)

            # group ACT functions (Exp x3, then Tanh, Ln last) to minimize
            # activation-table reloads
            u = ew("u")
            nc.vector.tensor_tensor(u[:], P1, P3, op=Alu.subtract)
            b_ = ew("b_")
            nc.scalar.activation(b_[:], P1, AF.Exp, scale=0.4)
            d_ = ew("d_")
            nc.scalar.activation(d_[:], P3, AF.Exp, scale=0.4)
            e = ew("e")
            nc.scalar.activation(e[:], u[:], AF.Exp, scale=0.4, bias=loga_sb[:])
            th = ew("th")
            nc.scalar.activation(th[:], P2, AF.Tanh, scale=0.3)
            c_ = ew("c_")
            nc.vector.tensor_mul(c_[:], th[:], e[:])
            t1 = ew("t1")
            nc.vector.tensor_mul(t1[:], d_[:], z2r_sb[:])
            arg = ew("arg")
            nc.vector.tensor_add(arg[:], t1[:], P4)
            sq = ew("sq")
            nc.vector.tensor_mul(sq[:], arg[:], arg[:])
            den = ew("den")
            nc.vector.tensor_scalar_add(den[:], sq[:], 1.0)
            rcp = ew("rcp")
            nc.vector.reciprocal(rcp[:], den[:])
            t2 = ew("t2")
            nc.vector.tensor_mul(t2[:], b_[:], z2r_sb[:])
            t3 = ew("t3")
            nc.vector.tensor_mul(t3[:], c_[:], rcp[:])
            s1 = ew("s1")
            nc.vector.tensor_add(s1[:], P0, t2[:])
            z2n_sb = ew("z2n_sb")
            nc.vector.tensor_add(z2n_sb[:], s1[:], t3[:])
            nc.sync.dma_start(d_z2n.ap(), z2n_sb[:])

            t4 = ew("t4")
            nc.vector.tensor_mul(t4[:], c_[:], d_[:])
            t5 = ew("t5")
            nc.vector.tensor_mul(t5[:], t4[:], arg[:])
            t6 = ew("t6")
            nc.vector.tensor_mul(t6[:], t5[:], rcp[:])
            t7 = ew("t7")
            nc.vector.tensor_mul(t7[:], t6[:], rcp[:])
            inner = ew("inner")
            nc.vector.scalar_tensor_tensor(
                inner[:], t7[:], -2.0, b_[:], op0=Alu.mult, op1=Alu.add
            )
            lg = ew("lg")
            nc.scalar.activation(lg[:], inner[:], AF.Ln)

            lg2 = mk(ewp, [128, NCHUNK], f32, "lg2")
            nc.vector.tensor_add(lg2[:], lg[:, :, 0], lg[:, :, 1])
            psl = mk(pp, [128, 512], f32, "ps")
            nc.tensor.matmul(
                psl[:NCHUNK, :4], lg2[:], mask_sb[:], start=True, stop=True
            )
            ld_sb = mk(ewp, [NCHUNK, 4], f32, "ld_sb")
            nc.vector.tensor_copy(ld_sb[:], psl[:NCHUNK, :4])
            nc.sync.dma_start(d_ld.ap(), ld_sb[:])

    nc.compile()
    return nc


def _get_program():
    if "nc" not in _CACHE:
        _CACHE["nc"] = _build_program()
    return _CACHE["nc"]


def _host_inputs(inputs):
    import ml_dtypes

    bf16 = ml_dtypes.bfloat16
    f32 = np.float32

    x = np.asarray(inputs["x"], f32)
    cond = np.asarray(inputs["cond"], f32)

    weT = np.ascontiguousarray(inputs["w_embed"][:, :2].T).astype(bf16)
    wc1T = np.ascontiguousarray(inputs["w_c1"].T).astype(bf16)
    wc2T = np.ascontiguousarray(inputs["w_c2"].T).reshape(4, 128, H).astype(bf16)
    wc0 = (
        np.ascontiguousarray(np.transpose(inputs["w_conv0"], (1, 2, 0)))
        .reshape(8, 128, 3, H)
        .astype(bf16)
    )
    wcv = np.stack(
        [
            np.ascontiguousarray(
                np.transpose(inputs[f"w_conv{i}"], (1, 2, 0))
            ).reshape(4, 128, 3, H)
            for i in (1, 2, 3, 4)
        ]
    ).astype(bf16)
    woutT = np.ascontiguousarray(inputs["w_out"].T).reshape(4, 128, 10).astype(bf16)
    bias_all = np.stack(
        [inputs["b_embed"], inputs["b_c1"], inputs["b_c2"]]
        + [inputs[f"b_conv{i}"] for i in range(5)]
    ).astype(f32)
    bias_pack = np.ascontiguousarray(
        bias_all.reshape(8, 4, 128).transpose(2, 0, 1)
    ).astype(f32)
    brep = np.ascontiguousarray(
        np.broadcast_to(inputs["b_out"].astype(f32), (128, NCHUNK, 10))
    )
    mask = np.zeros((128, 4), f32)
    mask[np.arange(128), np.arange(128) // 32] = 1.0

    in_maps = []
    for c in range(NCORES):
        xs = x[c * NB : (c + 1) * NB]
        z1 = xs[:, :T2]
        z2 = xs[:, T2:]
        z1T = np.ascontiguousarray(z1.reshape(NB * T2, 2).T).astype(bf16)
        z2r = np.ascontiguousarray(
            z2.reshape(NCHUNK, 4, T2, 2).transpose(1, 2, 0, 3)
        ).reshape(128, NCHUNK, 2)
        condT = np.ascontiguousarray(
            cond[c * NB : (c + 1) * NB].reshape(NB, 2 * COND).T
        ).astype(bf16)
        in_maps.append(
            dict(
                z1T=z1T,
                z2r=z2r,
                condT=condT,
                we=weT,
                wc1=wc1T,
                wc2=wc2T,
                wc0=wc0,
                wcv=wcv,
                wout=woutT,
                biases=bias_pack,
                brep=brep,
                mask=mask,
            )
        )
    return in_maps


def _assemble_output(x, results):
    z = np.empty((B, T, D), np.float32)
    ld = np.empty((B,), np.float32)
    for c in range(NCORES):
        z[c * NB : (c + 1) * NB, :T2] = x[c * NB : (c + 1) * NB, :T2]
        z2n = np.asarray(results[c]["z2n"], np.float32)
        z[c * NB : (c + 1) * NB, T2:] = (
            z2n.reshape(4, T2, NCHUNK, 2).transpose(2, 0, 1, 3).reshape(NB, T2, 2)
        )
        ld[c * NB : (c + 1) * NB] = np.asarray(results[c]["ld"], np.float32).reshape(
            NB
        )
    return z, ld


def run(inputs, trace=False, trace_cores=None):
    """Run on 8 NeuronCores; returns ((z, logdet), BassKernelResults)."""
    from concourse.bass_utils import run_bass_kernel_spmd

    nc = _get_program()
    in_maps = _host_inputs(inputs)
    res = run_bass_kernel_spmd(
        nc,
        in_maps,
        list(range(NCORES)),
        trace=trace,
        trace_cores=trace_cores if trace_cores is not None else list(range(NCORES)),
    )
    x = np.asarray(inputs["x"], np.float32)
    return _assemble_output(x, res.results), res


def kernel(**inputs):
    (z, ld), _ = run(inputs, trace=False)
    return z, ld


if __name__ == "__main__":
    print("build only:", _get_program())


# revision 20
# speedup vs baseline: 1.0059x; 1.0059x over previous
"""Trainium2 Bass kernel for nn_NlsqCond (ConvFlow NLSQ coupling layer).

Strategy: pure data parallel over batch B=256 -> 32 samples per core on 8
NeuronCores. Convs are computed as 3 shifted matmuls accumulating in PSUM
over a gap-column activation layout (34 columns per sample, zero guard
columns), so the k=3/pad=1 conv needs no boundary special-casing. Weights
are transposed host-side into lhsT layout and cast to bf16 (fp32 PSUM
accumulation); measured end-to-end error vs fp32 reference is ~1e-4.
The final projection is computed transposed ([cols, 10]) so the NLSQ
elementwise tail runs with full 128-partition parallelism; the per-sample
logdet partition-reduction is done with a small mask matmul.
"""

import math

import numpy as np

B, T, D, H, COND = 256, 64, 2, 512, 8
T2 = T // 2                      # 32
NCORES = 8
NB = B // NCORES                 # 32 samples per core
ST = T2 + 2                      # 34: per-sample column stride (zero gaps)
WCOLS = NB * ST                  # 1088
NG = 2                           # PSUM column groups per matmul set
GS = NB // NG                    # 16 samples per group
NCHUNK = NB * T2 // 128          # 8 column chunks of 128 for final proj
LOG_A = math.log(8.0 * math.sqrt(3.0) / 9.0 - 0.05)

_CACHE = {}


def _build_program():
    import concourse.bacc as bacc
    import concourse.mybir as mybir
    import concourse.tile as tile

    f32 = mybir.dt.float32
    bf16 = mybir.dt.bfloat16
    AF = mybir.ActivationFunctionType
    Alu = mybir.AluOpType

    nc = bacc.Bacc("TRN2", target_bir_lowering=False, debug=False)

    # ---- DRAM I/O ----
    d_z1T = nc.dram_tensor("z1T", [2, NB * T2], bf16, kind="ExternalInput")
    d_z2r = nc.dram_tensor("z2r", [128, NCHUNK, 2], f32, kind="ExternalInput")
    d_condT = nc.dram_tensor("condT", [2 * COND, NB], bf16, kind="ExternalInput")
    d_we = nc.dram_tensor("we", [2, H], bf16, kind="ExternalInput")
    d_wc1 = nc.dram_tensor("wc1", [2 * COND, H], bf16, kind="ExternalInput")
    # conv/linear weights packed partition-major so each DMA moves one large
    # contiguous chunk per partition (descriptor-rate, not bandwidth, limits
    # small-row DMAs)
    d_wc2 = nc.dram_tensor("wc2", [128, 4, H], bf16, kind="ExternalInput")
    d_wc0 = nc.dram_tensor("wc0", [128, 8, 3, H], bf16, kind="ExternalInput")
    d_wcv = nc.dram_tensor("wcv", [4, 128, 4, 3, H], bf16, kind="ExternalInput")
    d_wout = nc.dram_tensor("wout", [128, 4, 10], bf16, kind="ExternalInput")
    d_bias = nc.dram_tensor("biases", [128, 8, 4], f32, kind="ExternalInput")
    d_brep = nc.dram_tensor("brep", [128, NCHUNK, 10], f32, kind="ExternalInput")
    d_mask = nc.dram_tensor("mask", [128, 4], f32, kind="ExternalInput")
    d_z2n = nc.dram_tensor("z2n", [128, NCHUNK, 2], f32, kind="ExternalOutput")
    d_ld = nc.dram_tensor("ld", [NCHUNK, 4], f32, kind="ExternalOutput")

    with tile.TileContext(nc) as tc:
        with (
            tc.tile_pool(name="w", bufs=1) as wp,
            tc.tile_pool(name="act", bufs=1) as ap_,
            tc.tile_pool(name="ew", bufs=1) as ewp,
            tc.tile_pool(name="ps", bufs=8, space="PSUM") as pp,
        ):
            def mk(pool, shape, dtype, tag):
                return pool.tile(shape, dtype, tag=tag, name=tag)

            # ---- SBUF loads (small inputs first so PE can start early) ----
            z1T_sb = mk(ap_, [2, NB * T2], bf16, "z1T")
            nc.sync.dma_start(z1T_sb[:], d_z1T.ap())
            condT_sb = mk(ap_, [2 * COND, NB], bf16, "condT")
            nc.sync.dma_start(condT_sb[:], d_condT.ap())
            we_sb = mk(wp, [2, H], bf16, "we")
            nc.sync.dma_start(we_sb[:], d_we.ap())
            wc1_sb = mk(wp, [2 * COND, H], bf16, "wc1")
            nc.sync.dma_start(wc1_sb[:], d_wc1.ap())
            bias_sb = mk(wp, [128, 8, 4], f32, "bias")
            nc.sync.dma_start(bias_sb[:], d_bias.ap())
            wc2_sb = mk(wp, [128, 4, H], bf16, "wc2")
            nc.sync.dma_start(wc2_sb[:], d_wc2.ap())
            wc0_sb = mk(wp, [128, 8, 3, H], bf16, "wc0")
            nc.sync.dma_start(wc0_sb[:, 0:4], d_wc0.ap()[:, 0:4])
            nc.sync.dma_start(wc0_sb[:, 4:8], d_wc0.ap()[:, 4:8])
            wcv_sb = [mk(wp, [128, 4, 3, H], bf16, f"wcv_{l}") for l in range(4)]
            for l in range(4):
                nc.sync.dma_start(wcv_sb[l][:], d_wcv.ap()[l])
            wout_sb = mk(wp, [128, 4, 10], bf16, "wout")
            nc.sync.dma_start(wout_sb[:], d_wout.ap())
            brep_sb = mk(wp, [128, NCHUNK, 10], f32, "brep")
            nc.sync.dma_start(brep_sb[:], d_brep.ap())
            mask_sb = mk(wp, [128, 4], f32, "mask")
            nc.sync.dma_start(mask_sb[:], d_mask.ap())
            z2r_sb = mk(ewp, [128, NCHUNK, 2], f32, "z2r")
            nc.sync.dma_start(z2r_sb[:], d_z2r.ap())

            def bias_ap(idx, mt):
                return bias_sb[:, idx, mt : mt + 1]

            # ---- PE warm-up scratch (memset first so the PE can spin ASAP) ----
            wu_l = mk(ap_, [128, 128], bf16, "wu_l")
            wu_r = mk(ap_, [128, 512], bf16, "wu_r")
            nc.vector.memset(wu_l[:], 0.0)
            nc.vector.memset(wu_r[:], 0.0)

            # ---- activation ping-pong buffers (gap layout) ----
            # Only the gap guard columns need zeroing; valid columns are
            # always written before they are read.
            actA = [mk(ap_, [128, WCOLS], bf16, f"A{i}") for i in range(8)]
            actB = [mk(ap_, [128, WCOLS], bf16, f"B{i}") for i in range(4)]
            actC = [mk(ap_, [128, WCOLS], bf16, f"C{i}") for i in range(4)]
            for ti, t_ in enumerate(actA + actB + actC):
                v = t_[:].rearrange("p (s c) -> p s c", c=ST)
                eng = nc.vector if ti % 2 else nc.gpsimd
                eng.memset(v[:, :, 0:1], 0.0)
                eng.memset(v[:, :, ST - 1 : ST], 0.0)

            def valid(tl, g=None):
                v = tl[:].rearrange("p (s c) -> p s c", c=ST)
                if g is None:
                    return v[:, :, 1 : 1 + T2]
                return v[:, g * GS : (g + 1) * GS, 1 : 1 + T2]

            # ---- PE warm-up spin ----
            # The PE HAM clock gate starts at 1.2 GHz and only releases to
            # 2.4 GHz after ~3.4us of sustained activity. Matmul on scratch
            # zeros while the weight DMAs stream in, so the real conv stack
            # runs warm from its first instruction.
            ps_w = mk(pp, [128, 512], f32, "ps")
            for i in range(16):
                nc.tensor.matmul(
                    ps_w[:], wu_l[:], wu_r[:], start=(i == 0), stop=(i == 15)
                )

            # ---- embedding: h = z1 @ we.T + b_embed -> actA[0..3] ----
            for g in range(NG):
                for mt in range(4):
                    ps = mk(pp, [128, 512], f32, "ps")
                    nc.tensor.matmul(
                        ps[:],
                        we_sb[:, mt * 128 : (mt + 1) * 128],
                        z1T_sb[:, g * 512 : (g + 1) * 512],
                        start=True,
                        stop=True,
                    )
                    nc.scalar.activation(
                        valid(actA[mt], g),
                        ps[:].rearrange("p (s t) -> p s t", t=T2),
                        AF.Identity,
                        bias=bias_ap(0, mt),
                    )

            # ---- cond MLP: c2 = relu(W2 relu(W1 c + b1) + b2) ----
            c1_sb = [mk(ap_, [128, NB], bf16, f"c1_{i}") for i in range(4)]
            c2_sb = [mk(ap_, [128, NB], bf16, f"c2_{i}") for i in range(4)]
            for mt in range(4):
                ps = mk(pp, [128, 512], f32, "ps")
                nc.tensor.matmul(
                    ps[:, :NB],
                    wc1_sb[:, mt * 128 : (mt + 1) * 128],
                    condT_sb[:],
                    start=True,
                    stop=True,
                )
                nc.scalar.activation(
                    c1_sb[mt][:], ps[:, :NB], AF.Relu, bias=bias_ap(1, mt)
                )
            for mt in range(4):
                ps = mk(pp, [128, 512], f32, "ps")
                for kt in range(4):
                    nc.tensor.matmul(
                        ps[:, :NB],
                        wc2_sb[:, kt, mt * 128 : (mt + 1) * 128],
                        c1_sb[kt][:],
                        start=(kt == 0),
                        stop=(kt == 3),
                    )
                nc.scalar.activation(
                    c2_sb[mt][:], ps[:, :NB], AF.Relu, bias=bias_ap(2, mt)
                )
            # broadcast c2 over positions t -> actA[4..7]
            for mt in range(4):
                src = c2_sb[mt][:].unsqueeze(2).broadcast_to([128, NB, T2])
                nc.scalar.copy(valid(actA[4 + mt]), src)

            # ---- conv stack ----
            # Matmul moving operands must be single-free-dim, so each conv
            # matmul streams a contiguous window of the gap layout; outputs
            # at gap positions are garbage and simply never read back.
            WINDOWS = [(0, 15), (15, 15), (30, 2)]  # (sample base, n samples)
            srcs, ktn, wts = actA, 8, wc0_sb[:]
            for L in range(5):
                dsts = actB if L % 2 == 0 else actC
                for mt in range(4):
                    pss = [mk(pp, [128, 512], f32, "ps") for _ in WINDOWS]
                    nacc = 3 * ktn
                    i = 0
                    # kt outer: matches weight-DMA arrival order, and for
                    # conv0 defers the cond-broadcast k-tiles (4..7) so the
                    # broadcast copies have more slack
                    for kt in range(ktn):
                        for k in range(3):
                            lhsT = wts[:, kt, k, mt * 128 : (mt + 1) * 128]
                            for wi, (sb, ns) in enumerate(WINDOWS):
                                n = ns * ST - 2
                                base = sb * ST + k
                                nc.tensor.matmul(
                                    pss[wi][:, :n],
                                    lhsT,
                                    srcs[kt][:, base : base + n],
                                    start=(i == 0),
                                    stop=(i == nacc - 1),
                                )
                            i += 1
                    for wi, (sb, ns) in enumerate(WINDOWS):
                        pv_ = pss[wi][:, : ns * ST].rearrange(
                            "p (s c) -> p s c", c=ST
                        )[:, :, 0:T2]
                        dv_ = dsts[mt][:].rearrange("p (s c) -> p s c", c=ST)[
                            :, sb : sb + ns, 1 : 1 + T2
                        ]
                        # split the PSUM->SBUF relu copies across ACT and DVE
                        if (mt * 3 + wi) % 2 == 0:
                            nc.scalar.activation(
                                dv_, pv_, AF.Relu, bias=bias_ap(3 + L, mt)
                            )
                        else:
                            nc.vector.tensor_scalar(
                                dv_,
                                pv_,
                                bias_ap(3 + L, mt),
                                0.0,
                                Alu.add,
                                Alu.max,
                            )
                srcs, ktn = dsts, 4
                if L < 4:
                    wts = wcv_sb[L][:]

            # ---- compact the final activation (drop gap columns) ----
            h5 = [mk(ap_, [128, NB * T2], bf16, f"h5_{i}") for i in range(4)]
            for kt in range(4):
                eng = nc.scalar if kt % 2 == 0 else nc.vector
                if kt % 2 == 0:
                    nc.scalar.copy(
                        h5[kt][:].rearrange("p (s t) -> p s t", t=T2),
                        valid(srcs[kt]),
                    )
                else:
                    nc.vector.tensor_copy(
                        h5[kt][:].rearrange("p (s t) -> p s t", t=T2),
                        valid(srcs[kt]),
                    )

            # ---- final projection (transposed): out[col, 10] ----
            pso = mk(pp, [128, 512], f32, "ps")
            for j in range(NCHUNK):
                for kt in range(4):
                    nc.tensor.matmul(
                        pso[:, j * 10 : (j + 1) * 10],
                        h5[kt][:, j * 128 : (j + 1) * 128],
                        wout_sb[:, kt],
                        start=(kt == 0),
                        stop=(kt == 3),
                    )

            # ---- NLSQ elementwise tail ----
            def ew(tag):
                return mk(ewp, [128, NCHUNK, 2], f32, tag)

            params = mk(ewp, [128, NCHUNK, 10], f32, "params")
            nc.vector.tensor_add(
                params[:],
                pso[:, : NCHUNK * 10].rearrange("p (j q) -> p j q", q=10),
                brep_sb[:],
            )
            pv = params[:].rearrange("p j (a q) -> p j a q", q=5)
            P0, P1, P2, P3, P4 = (pv[:, :, :, i] for i in range(5))

            loga_sb = mk(ewp, [128, 1], f32, "loga")
            nc.vector.memset(loga_sb[:], LOG_A)

            # group ACT functions (Exp x3, then Tanh, Ln last) to minimize
            # activation-table reloads
            u = ew("u")
            nc.vector.tensor_tensor(u[:], P1, P3, op=Alu.subtract)
            b_ = ew("b_")
            nc.scalar.activation(b_[:], P1, AF.Exp, scale=0.4)
            d_ = ew("d_")
            nc.scalar.activation(d_[:], P3, AF.Exp, scale=0.4)
            e = ew("e")
            nc.scalar.activation(e[:], u[:], AF.Exp, scale=0.4, bias=loga_sb[:])
            th = ew("th")
            nc.scalar.activation(th[:], P2, AF.Tanh, scale=0.3)
            c_ = ew("c_")
            nc.vector.tensor_mul(c_[:], th[:], e[:])
            t1 = ew("t1")
            nc.vector.tensor_mul(t1[:], d_[:], z2r_sb[:])
            arg = ew("arg")
            nc.vector.tensor_add(arg[:], t1[:], P4)
            sq = ew("sq")
            nc.vector.tensor_mul(sq[:], arg[:], arg[:])
            den = ew("den")
            nc.vector.tensor_scalar_add(den[:], sq[:], 1.0)
            rcp = ew("rcp")
            nc.vector.reciprocal(rcp[:], den[:])
            t2 = ew("t2")
            nc.vector.tensor_mul(t2[:], b_[:], z2r_sb[:])
            t3 = ew("t3")
            nc.vector.tensor_mul(t3[:], c_[:], rcp[:])
            s1 = ew("s1")
            nc.vector.tensor_add(s1[:], P0, t2[:])
            z2n_sb = ew("z2n_sb")
            nc.vector.tensor_add(z2n_sb[:], s1[:], t3[:])
            nc.sync.dma_start(d_z2n.ap(), z2n_sb[:])

            t4 = ew("t4")
            nc.vector.tensor_mul(t4[:], c_[:], d_[:])
            t5 = ew("t5")
            nc.vector.tensor_mul(t5[:], t4[:], arg[:])
            t6 = ew("t6")
            nc.vector.tensor_mul(t6[:], t5[:], rcp[:])
            t7 = ew("t7")
            nc.vector.tensor_mul(t7[:], t6[:], rcp[:])
            inner = ew("inner")
            nc.vector.scalar_tensor_tensor(
                inner[:], t7[:], -2.0, b_[:], op0=Alu.mult, op1=Alu.add
            )
            lg = ew("lg")
            nc.scalar.activation(lg[:], inner[:], AF.Ln)

            lg2 = mk(ewp, [128, NCHUNK], f32, "lg2")
            nc.vector.tensor_add(lg2[:], lg[:, :, 0], lg[:, :, 1])
            psl = mk(pp, [128, 512], f32, "ps")
            nc.tensor.matmul(
                psl[:NCHUNK, :4], lg2[:], mask_sb[:], start=True, stop=True
            )
            ld_sb = mk(ewp, [NCHUNK, 4], f32, "ld_sb")
            nc.vector.tensor_copy(ld_sb[:], psl[:NCHUNK, :4])
            nc.sync.dma_start(d_ld.ap(), ld_sb[:])

    nc.compile()
    return nc


def _get_program():
    if "nc" not in _CACHE:
        _CACHE["nc"] = _build_program()
    return _CACHE["nc"]


def _host_inputs(inputs):
    import ml_dtypes

    bf16 = ml_dtypes.bfloat16
    f32 = np.float32

    x = np.asarray(inputs["x"], f32)
    cond = np.asarray(inputs["cond"], f32)

    weT = np.ascontiguousarray(inputs["w_embed"][:, :2].T).astype(bf16)
    wc1T = np.ascontiguousarray(inputs["w_c1"].T).astype(bf16)
    # partition-major packing: [128, ...] with large contiguous per-partition
    # chunks so SBUF DMAs are few big descriptors per partition
    wc2T = np.ascontiguousarray(
        inputs["w_c2"].T.reshape(4, 128, H).transpose(1, 0, 2)
    ).astype(bf16)
    wc0 = np.ascontiguousarray(
        np.transpose(inputs["w_conv0"], (1, 2, 0))
        .reshape(8, 128, 3, H)
        .transpose(1, 0, 2, 3)
    ).astype(bf16)
    wcv = np.stack(
        [
            np.ascontiguousarray(
                np.transpose(inputs[f"w_conv{i}"], (1, 2, 0))
                .reshape(4, 128, 3, H)
                .transpose(1, 0, 2, 3)
            )
            for i in (1, 2, 3, 4)
        ]
    ).astype(bf16)
    woutT = np.ascontiguousarray(
        inputs["w_out"].T.reshape(4, 128, 10).transpose(1, 0, 2)
    ).astype(bf16)
    bias_all = np.stack(
        [inputs["b_embed"], inputs["b_c1"], inputs["b_c2"]]
        + [inputs[f"b_conv{i}"] for i in range(5)]
    ).astype(f32)
    bias_pack = np.ascontiguousarray(
        bias_all.reshape(8, 4, 128).transpose(2, 0, 1)
    ).astype(f32)
    brep = np.ascontiguousarray(
        np.broadcast_to(inputs["b_out"].astype(f32), (128, NCHUNK, 10))
    )
    mask = np.zeros((128, 4), f32)
    mask[np.arange(128), np.arange(128) // 32] = 1.0

    in_maps = []
    for c in range(NCORES):
        xs = x[c * NB : (c + 1) * NB]
        z1 = xs[:, :T2]
        z2 = xs[:, T2:]
        z1T = np.ascontiguousarray(z1.reshape(NB * T2, 2).T).astype(bf16)
        z2r = np.ascontiguousarray(
            z2.reshape(NCHUNK, 4, T2, 2).transpose(1, 2, 0, 3)
        ).reshape(128, NCHUNK, 2)
        condT = np.ascontiguousarray(
            cond[c * NB : (c + 1) * NB].reshape(NB, 2 * COND).T
        ).astype(bf16)
        in_maps.append(
            dict(
                z1T=z1T,
                z2r=z2r,
                condT=condT,
                we=weT,
                wc1=wc1T,
                wc2=wc2T,
                wc0=wc0,
                wcv=wcv,
                wout=woutT,
                biases=bias_pack,
                brep=brep,
                mask=mask,
            )
        )
    return in_maps


def _assemble_output(x, results):
    z = np.empty((B, T, D), np.float32)
    ld = np.empty((B,), np.float32)
    for c in range(NCORES):
        z[c * NB : (c + 1) * NB, :T2] = x[c * NB : (c + 1) * NB, :T2]
        z2n = np.asarray(results[c]["z2n"], np.float32)
        z[c * NB : (c + 1) * NB, T2:] = (
            z2n.reshape(4, T2, NCHUNK, 2).transpose(2, 0, 1, 3).reshape(NB, T2, 2)
        )
        ld[c * NB : (c + 1) * NB] = np.asarray(results[c]["ld"], np.float32).reshape(
            NB
        )
    return z, ld


def run(inputs, trace=False, trace_cores=None):
    """Run on 8 NeuronCores; returns ((z, logdet), BassKernelResults)."""
    from concourse.bass_utils import run_bass_kernel_spmd

    nc = _get_program()
    in_maps = _host_inputs(inputs)
    res = run_bass_kernel_spmd(
        nc,
        in_maps,
        list(range(NCORES)),
        trace=trace,
        trace_cores=trace_cores if trace_cores is not None else list(range(NCORES)),
    )
    x = np.asarray(inputs["x"], np.float32)
    return _assemble_output(x, res.results), res


def kernel(**inputs):
    (z, ld), _ = run(inputs, trace=False)
    return z, ld


if __name__ == "__main__":
    print("build only:", _get_program())


# revision 28
# speedup vs baseline: 1.2221x; 1.2149x over previous
"""Trainium2 Bass kernel for nn_NlsqCond (ConvFlow NLSQ coupling layer).

Strategy: pure data parallel over batch B=256 -> 32 samples per core on 8
NeuronCores. Convs are computed as 3 shifted matmuls accumulating in PSUM
over a gap-column activation layout (34 columns per sample, zero guard
columns), so the k=3/pad=1 conv needs no boundary special-casing. Weights
are transposed host-side into lhsT layout and cast to bf16 (fp32 PSUM
accumulation); measured end-to-end error vs fp32 reference is ~1e-4.
The final projection is computed transposed ([cols, 10]) so the NLSQ
elementwise tail runs with full 128-partition parallelism; the per-sample
logdet partition-reduction is done with a small mask matmul.
"""

import math

import numpy as np

B, T, D, H, COND = 256, 64, 2, 512, 8
T2 = T // 2                      # 32
NCORES = 8
NB = B // NCORES                 # 32 samples per core
ST = T2 + 2                      # 34: per-sample column stride (zero gaps)
WCOLS = NB * ST                  # 1088
NG = 2                           # PSUM column groups per matmul set
GS = NB // NG                    # 16 samples per group
NCHUNK = NB * T2 // 128          # 8 column chunks of 128 for final proj
LOG_A = math.log(8.0 * math.sqrt(3.0) / 9.0 - 0.05)

_CACHE = {}


def _build_program():
    import concourse.bacc as bacc
    import concourse.mybir as mybir
    import concourse.tile as tile

    f32 = mybir.dt.float32
    bf16 = mybir.dt.bfloat16
    AF = mybir.ActivationFunctionType
    Alu = mybir.AluOpType

    nc = bacc.Bacc("TRN2", target_bir_lowering=False, debug=False)

    # ---- DRAM I/O ----
    # z1 in gap layout (zero guard columns), feeds conv0's rank-2 h-half
    d_z1g = nc.dram_tensor("z1g", [2, WCOLS], bf16, kind="ExternalInput")
    d_z2r = nc.dram_tensor("z2r", [128, NCHUNK, 2], f32, kind="ExternalInput")
    d_condT = nc.dram_tensor("condT", [2 * COND, NB], bf16, kind="ExternalInput")
    d_wc1 = nc.dram_tensor("wc1", [2 * COND, H], bf16, kind="ExternalInput")
    # conv/linear weights packed partition-major so each DMA moves one large
    # contiguous chunk per partition (descriptor-rate, not bandwidth, limits
    # small-row DMAs)
    d_wc2 = nc.dram_tensor("wc2", [128, 4, H], bf16, kind="ExternalInput")
    # conv0 h-half collapsed to rank 2: wh2[k] = (w_conv0[:, :H, k] @ we).T
    d_wh2 = nc.dram_tensor("wh2", [2, 3, H], bf16, kind="ExternalInput")
    # conv0 cond-half collapsed to per-sample vectors: types (sum_k, k=0, k=2)
    d_qw = nc.dram_tensor("qw", [128, 4, 3, H], bf16, kind="ExternalInput")
    # b_embed fold rank-1 terms per type
    d_vb = nc.dram_tensor("vb", [1, 3, H], bf16, kind="ExternalInput")
    d_wcv = nc.dram_tensor("wcv", [4, 128, 4, 3, H], bf16, kind="ExternalInput")
    d_wout = nc.dram_tensor("wout", [128, 4, 10], bf16, kind="ExternalInput")
    d_bias = nc.dram_tensor("biases", [128, 8, 4], f32, kind="ExternalInput")
    d_brep = nc.dram_tensor("brep", [128, NCHUNK, 10], f32, kind="ExternalInput")
    d_mask = nc.dram_tensor("mask", [128, 4], f32, kind="ExternalInput")
    d_z2n = nc.dram_tensor("z2n", [128, NCHUNK, 2], f32, kind="ExternalOutput")
    d_ld = nc.dram_tensor("ld", [NCHUNK, 4], f32, kind="ExternalOutput")

    with tile.TileContext(nc) as tc:
        with (
            tc.tile_pool(name="w", bufs=1) as wp,
            tc.tile_pool(name="act", bufs=1) as ap_,
            tc.tile_pool(name="ew", bufs=1) as ewp,
            tc.tile_pool(name="ps", bufs=8, space="PSUM") as pp,
        ):
            def mk(pool, shape, dtype, tag):
                return pool.tile(shape, dtype, tag=tag, name=tag)

            # ---- SBUF loads (small inputs first so PE can start early) ----
            z1g_sb = mk(ap_, [2, WCOLS], bf16, "z1g")
            nc.sync.dma_start(z1g_sb[:], d_z1g.ap())
            condT_sb = mk(ap_, [2 * COND, NB], bf16, "condT")
            nc.sync.dma_start(condT_sb[:], d_condT.ap())
            wh2_sb = mk(wp, [2, 3, H], bf16, "wh2")
            nc.sync.dma_start(wh2_sb[:], d_wh2.ap())
            wc1_sb = mk(wp, [2 * COND, H], bf16, "wc1")
            nc.sync.dma_start(wc1_sb[:], d_wc1.ap())
            vb_sb = mk(wp, [1, 3, H], bf16, "vb")
            nc.sync.dma_start(vb_sb[:], d_vb.ap())
            bias_sb = mk(wp, [128, 8, 4], f32, "bias")
            nc.sync.dma_start(bias_sb[:], d_bias.ap())
            wc2_sb = mk(wp, [128, 4, H], bf16, "wc2")
            nc.sync.dma_start(wc2_sb[:], d_wc2.ap())
            qw_sb = mk(wp, [128, 4, 3, H], bf16, "qw")
            nc.sync.dma_start(qw_sb[:], d_qw.ap())
            wcv_sb = [mk(wp, [128, 4, 3, H], bf16, f"wcv_{l}") for l in range(4)]
            for l in range(4):
                nc.sync.dma_start(wcv_sb[l][:], d_wcv.ap()[l])
            wout_sb = mk(wp, [128, 4, 10], bf16, "wout")
            nc.sync.dma_start(wout_sb[:], d_wout.ap())
            brep_sb = mk(wp, [128, NCHUNK, 10], f32, "brep")
            nc.sync.dma_start(brep_sb[:], d_brep.ap())
            mask_sb = mk(wp, [128, 4], f32, "mask")
            nc.sync.dma_start(mask_sb[:], d_mask.ap())
            z2r_sb = mk(ewp, [128, NCHUNK, 2], f32, "z2r")
            nc.sync.dma_start(z2r_sb[:], d_z2r.ap())

            def bias_ap(idx, mt):
                return bias_sb[:, idx, mt : mt + 1]

            # ---- PE warm-up scratch (memset first so the PE can spin ASAP) ----
            wu_l = mk(ap_, [128, 128], bf16, "wu_l")
            wu_r = mk(ap_, [128, 512], bf16, "wu_r")
            nc.vector.memset(wu_l[:], 0.0)
            nc.vector.memset(wu_r[:], 0.0)

            # ---- activation ping-pong buffers (gap layout) ----
            # Only the gap guard columns need zeroing; valid columns are
            # always written before they are read.
            actB = [mk(ap_, [128, WCOLS], bf16, f"B{i}") for i in range(4)]
            actC = [mk(ap_, [128, WCOLS], bf16, f"C{i}") for i in range(4)]
            for ti, t_ in enumerate(actB + actC):
                v = t_[:].rearrange("p (s c) -> p s c", c=ST)
                eng = nc.vector if ti % 2 else nc.gpsimd
                eng.memset(v[:, :, 0:1], 0.0)
                eng.memset(v[:, :, ST - 1 : ST], 0.0)
            ones_sb = mk(ap_, [1, NB], bf16, "ones")
            nc.vector.memset(ones_sb[:], 1.0)

            def valid(tl, g=None):
                v = tl[:].rearrange("p (s c) -> p s c", c=ST)
                if g is None:
                    return v[:, :, 1 : 1 + T2]
                return v[:, g * GS : (g + 1) * GS, 1 : 1 + T2]

            # ---- PE warm-up spin ----
            # The PE HAM clock gate starts at 1.2 GHz and only releases to
            # 2.4 GHz after ~3.4us of sustained activity. Matmul on scratch
            # zeros while the weight DMAs stream in, so the real conv stack
            # runs warm from its first instruction.
            ps_w = mk(pp, [128, 512], f32, "ps")
            for i in range(16):
                nc.tensor.matmul(
                    ps_w[:], wu_l[:], wu_r[:], start=(i == 0), stop=(i == 15)
                )

            # ---- cond MLP: c2 = relu(W2 relu(W1 c + b1) + b2) ----
            c1_sb = [mk(ap_, [128, NB], bf16, f"c1_{i}") for i in range(4)]
            c2_sb = [mk(ap_, [128, NB], bf16, f"c2_{i}") for i in range(4)]
            for mt in range(4):
                ps = mk(pp, [128, 512], f32, "ps")
                nc.tensor.matmul(
                    ps[:, :NB],
                    wc1_sb[:, mt * 128 : (mt + 1) * 128],
                    condT_sb[:],
                    start=True,
                    stop=True,
                )
                nc.scalar.activation(
                    c1_sb[mt][:], ps[:, :NB], AF.Relu, bias=bias_ap(1, mt)
                )
            for mt in range(4):
                ps = mk(pp, [128, 512], f32, "ps")
                for kt in range(4):
                    nc.tensor.matmul(
                        ps[:, :NB],
                        wc2_sb[:, kt, mt * 128 : (mt + 1) * 128],
                        c1_sb[kt][:],
                        start=(kt == 0),
                        stop=(kt == 3),
                    )
                nc.scalar.activation(
                    c2_sb[mt][:], ps[:, :NB], AF.Relu, bias=bias_ap(2, mt)
                )

            # ---- conv0 cond-half collapsed to per-sample vectors ----
            # q[:, 0, s] = (sum_k W0k_c) @ c2[s] + sum_k(W0k_h @ b_embed)
            # q[:, 1, s] = W00_c @ c2[s] + W00_h @ b_embed   (t=0 correction)
            # q[:, 2, s] = W02_c @ c2[s] + W02_h @ b_embed   (t=31 correction)
            q_sb = []
            for mt in range(4):
                ps = mk(pp, [128, 512], f32, "ps")
                for ty in range(3):
                    for kt in range(4):
                        nc.tensor.matmul(
                            ps[:, ty * NB : (ty + 1) * NB],
                            qw_sb[:, kt, ty, mt * 128 : (mt + 1) * 128],
                            c2_sb[kt][:],
                            start=(kt == 0),
                            stop=False,
                        )
                    nc.tensor.matmul(
                        ps[:, ty * NB : (ty + 1) * NB],
                        vb_sb[:, ty, mt * 128 : (mt + 1) * 128],
                        ones_sb[:],
                        start=False,
                        stop=True,
                    )
                q = mk(ewp, [128, 3, NB], f32, f"q_{mt}")
                nc.vector.tensor_copy(
                    q[:], ps[:, : 3 * NB].rearrange("p (y s) -> p y s", s=NB)
                )
                q_sb.append(q)

            # ---- conv stack ----
            # Matmul moving operands must be single-free-dim, so each conv
            # matmul streams a contiguous window of the gap layout; outputs
            # at gap positions are garbage and simply never read back.
            WINDOWS = [(0, 15), (15, 15), (30, 2)]  # (sample base, n samples)
            for L in range(5):
                srcs = [actB, actC, actB, actC][L - 1] if L > 0 else None
                ktn = 4 if L > 0 else 1
                wts = wcv_sb[L - 1][:] if L > 0 else None
                dsts = actB if L % 2 == 0 else actC
                for mt in range(4):
                    pss = [mk(pp, [128, 512], f32, "ps") for _ in WINDOWS]
                    nacc = 3 * ktn
                    i = 0
                    # kt outer: matches weight-DMA arrival order
                    for kt in range(ktn):
                        for k in range(3):
                            if L == 0:
                                lhsT = wh2_sb[:, k, mt * 128 : (mt + 1) * 128]
                            else:
                                lhsT = wts[:, kt, k, mt * 128 : (mt + 1) * 128]
                            for wi, (sb, ns) in enumerate(WINDOWS):
                                n = ns * ST - 2
                                base = sb * ST + k
                                src = (
                                    z1g_sb[:, base : base + n]
                                    if L == 0
                                    else srcs[kt][:, base : base + n]
                                )
                                nc.tensor.matmul(
                                    pss[wi][:, :n],
                                    lhsT,
                                    src,
                                    start=(i == 0),
                                    stop=(i == nacc - 1),
                                )
                            i += 1
                    for wi, (sb, ns) in enumerate(WINDOWS):
                        pv3 = pss[wi][:, : ns * ST].rearrange(
                            "p (s c) -> p s c", c=ST
                        )
                        if L == 0:
                            # add broadcast cond term + boundary corrections
                            qv = q_sb[mt][:, 0, sb : sb + ns]
                            nc.vector.tensor_add(
                                pv3[:, :, 0:T2],
                                pv3[:, :, 0:T2],
                                qv.unsqueeze(2).broadcast_to([128, ns, T2]),
                            )
                            nc.vector.tensor_sub(
                                pv3[:, :, 0:1],
                                pv3[:, :, 0:1],
                                q_sb[mt][:, 1, sb : sb + ns].unsqueeze(2),
                            )
                            nc.vector.tensor_sub(
                                pv3[:, :, T2 - 1 : T2],
                                pv3[:, :, T2 - 1 : T2],
                                q_sb[mt][:, 2, sb : sb + ns].unsqueeze(2),
                            )
                        pv_ = pv3[:, :, 0:T2]
                        dv_ = dsts[mt][:].rearrange("p (s c) -> p s c", c=ST)[
                            :, sb : sb + ns, 1 : 1 + T2
                        ]
                        # split the PSUM->SBUF relu copies across ACT and DVE
                        if (mt * 3 + wi) % 2 == 0:
                            nc.scalar.activation(
                                dv_, pv_, AF.Relu, bias=bias_ap(3 + L, mt)
                            )
                        else:
                            nc.vector.tensor_scalar(
                                dv_,
                                pv_,
                                bias_ap(3 + L, mt),
                                0.0,
                                Alu.add,
                                Alu.max,
                            )
            srcs = actB

            # ---- compact the final activation (drop gap columns) ----
            h5 = [mk(ap_, [128, NB * T2], bf16, f"h5_{i}") for i in range(4)]
            for kt in range(4):
                eng = nc.scalar if kt % 2 == 0 else nc.vector
                if kt % 2 == 0:
                    nc.scalar.copy(
                        h5[kt][:].rearrange("p (s t) -> p s t", t=T2),
                        valid(srcs[kt]),
                    )
                else:
                    nc.vector.tensor_copy(
                        h5[kt][:].rearrange("p (s t) -> p s t", t=T2),
                        valid(srcs[kt]),
                    )

            # ---- final projection (transposed): out[col, 10] ----
            pso = mk(pp, [128, 512], f32, "ps")
            for j in range(NCHUNK):
                for kt in range(4):
                    nc.tensor.matmul(
                        pso[:, j * 10 : (j + 1) * 10],
                        h5[kt][:, j * 128 : (j + 1) * 128],
                        wout_sb[:, kt],
                        start=(kt == 0),
                        stop=(kt == 3),
                    )

            # ---- NLSQ elementwise tail ----
            def ew(tag):
                return mk(ewp, [128, NCHUNK, 2], f32, tag)

            params = mk(ewp, [128, NCHUNK, 10], f32, "params")
            nc.vector.tensor_add(
                params[:],
                pso[:, : NCHUNK * 10].rearrange("p (j q) -> p j q", q=10),
                brep_sb[:],
            )
            pv = params[:].rearrange("p j (a q) -> p j a q", q=5)
            P0, P1, P2, P3, P4 = (pv[:, :, :, i] for i in range(5))

            loga_sb = mk(ewp, [128, 1], f32, "loga")
            nc.vector.memset(loga_sb[:], LOG_A)

            # group ACT functions (Exp x3, then Tanh, Ln last) to minimize
            # activation-table reloads
            u = ew("u")
            nc.vector.tensor_tensor(u[:], P1, P3, op=Alu.subtract)
            b_ = ew("b_")
            nc.scalar.activation(b_[:], P1, AF.Exp, scale=0.4)
            d_ = ew("d_")
            nc.scalar.activation(d_[:], P3, AF.Exp, scale=0.4)
            e = ew("e")
            nc.scalar.activation(e[:], u[:], AF.Exp, scale=0.4, bias=loga_sb[:])
            th = ew("th")
            nc.scalar.activation(th[:], P2, AF.Tanh, scale=0.3)
            c_ = ew("c_")
            nc.vector.tensor_mul(c_[:], th[:], e[:])
            t1 = ew("t1")
            nc.vector.tensor_mul(t1[:], d_[:], z2r_sb[:])
            arg = ew("arg")
            nc.vector.tensor_add(arg[:], t1[:], P4)
            sq = ew("sq")
            nc.vector.tensor_mul(sq[:], arg[:], arg[:])
            den = ew("den")
            nc.vector.tensor_scalar_add(den[:], sq[:], 1.0)
            rcp = ew("rcp")
            nc.vector.reciprocal(rcp[:], den[:])
            t2 = ew("t2")
            nc.vector.tensor_mul(t2[:], b_[:], z2r_sb[:])
            t3 = ew("t3")
            nc.vector.tensor_mul(t3[:], c_[:], rcp[:])
            s1 = ew("s1")
            nc.vector.tensor_add(s1[:], P0, t2[:])
            z2n_sb = ew("z2n_sb")
            nc.vector.tensor_add(z2n_sb[:], s1[:], t3[:])
            nc.sync.dma_start(d_z2n.ap(), z2n_sb[:])

            t4 = ew("t4")
            nc.vector.tensor_mul(t4[:], c_[:], d_[:])
            t5 = ew("t5")
            nc.vector.tensor_mul(t5[:], t4[:], arg[:])
            t6 = ew("t6")
            nc.vector.tensor_mul(t6[:], t5[:], rcp[:])
            t7 = ew("t7")
            nc.vector.tensor_mul(t7[:], t6[:], rcp[:])
            inner = ew("inner")
            nc.vector.scalar_tensor_tensor(
                inner[:], t7[:], -2.0, b_[:], op0=Alu.mult, op1=Alu.add
            )
            lg = ew("lg")
            nc.scalar.activation(lg[:], inner[:], AF.Ln)

            lg2 = mk(ewp, [128, NCHUNK], f32, "lg2")
            nc.vector.tensor_add(lg2[:], lg[:, :, 0], lg[:, :, 1])
            psl = mk(pp, [128, 512], f32, "ps")
            nc.tensor.matmul(
                psl[:NCHUNK, :4], lg2[:], mask_sb[:], start=True, stop=True
            )
            ld_sb = mk(ewp, [NCHUNK, 4], f32, "ld_sb")
            nc.vector.tensor_copy(ld_sb[:], psl[:NCHUNK, :4])
            nc.sync.dma_start(d_ld.ap(), ld_sb[:])

    nc.compile()
    return nc


def _get_program():
    if "nc" not in _CACHE:
        _CACHE["nc"] = _build_program()
    return _CACHE["nc"]


def _host_inputs(inputs):
    import ml_dtypes

    bf16 = ml_dtypes.bfloat16
    f32 = np.float32

    x = np.asarray(inputs["x"], f32)
    cond = np.asarray(inputs["cond"], f32)

    wc1T = np.ascontiguousarray(inputs["w_c1"].T).astype(bf16)
    # partition-major packing: [128, ...] with large contiguous per-partition
    # chunks so SBUF DMAs are few big descriptors per partition
    wc2T = np.ascontiguousarray(
        inputs["w_c2"].T.reshape(4, 128, H).transpose(1, 0, 2)
    ).astype(bf16)
    # conv0 h-half collapsed through the rank-2 embedding
    w0 = np.asarray(inputs["w_conv0"], f32)
    we2 = np.asarray(inputs["w_embed"], f32)[:, :2]
    wh2 = np.ascontiguousarray(
        np.einsum("ock,cd->dko", w0[:, :H, :], we2)
    ).astype(bf16)
    # conv0 cond-half collapsed to per-sample vectors (types: sum_k, k0, k2)
    S = w0[:, H:, :]
    mats = np.stack([(S[:, :, 0] + S[:, :, 1] + S[:, :, 2]).T, S[:, :, 0].T, S[:, :, 2].T])
    qw = np.ascontiguousarray(
        mats.reshape(3, 4, 128, H).transpose(2, 1, 0, 3)
    ).astype(bf16)
    # b_embed folded through conv0 (rank-1 terms per type)
    be = np.asarray(inputs["b_embed"], f32)
    v = np.stack([w0[:, :H, k] @ be for k in range(3)])  # [3, H(out)]
    vb = np.ascontiguousarray(
        np.stack([v[0] + v[1] + v[2], v[0], v[2]])[None, :, :]
    ).astype(bf16)
    wcv = np.stack(
        [
            np.ascontiguousarray(
                np.transpose(inputs[f"w_conv{i}"], (1, 2, 0))
                .reshape(4, 128, 3, H)
                .transpose(1, 0, 2, 3)
            )
            for i in (1, 2, 3, 4)
        ]
    ).astype(bf16)
    woutT = np.ascontiguousarray(
        inputs["w_out"].T.reshape(4, 128, 10).transpose(1, 0, 2)
    ).astype(bf16)
    bias_all = np.stack(
        [inputs["b_embed"], inputs["b_c1"], inputs["b_c2"]]
        + [inputs[f"b_conv{i}"] for i in range(5)]
    ).astype(f32)
    bias_pack = np.ascontiguousarray(
        bias_all.reshape(8, 4, 128).transpose(2, 0, 1)
    ).astype(f32)
    brep = np.ascontiguousarray(
        np.broadcast_to(inputs["b_out"].astype(f32), (128, NCHUNK, 10))
    )
    mask = np.zeros((128, 4), f32)
    mask[np.arange(128), np.arange(128) // 32] = 1.0

    in_maps = []
    for c in range(NCORES):
        xs = x[c * NB : (c + 1) * NB]
        z1 = xs[:, :T2]
        z2 = xs[:, T2:]
        z1g = np.zeros((2, NB, ST), np.float32)
        z1g[:, :, 1 : 1 + T2] = z1.transpose(2, 0, 1)
        z1g = z1g.reshape(2, WCOLS).astype(bf16)
        z2r = np.ascontiguousarray(
            z2.reshape(NCHUNK, 4, T2, 2).transpose(1, 2, 0, 3)
        ).reshape(128, NCHUNK, 2)
        condT = np.ascontiguousarray(
            cond[c * NB : (c + 1) * NB].reshape(NB, 2 * COND).T
        ).astype(bf16)
        in_maps.append(
            dict(
                z1g=z1g,
                z2r=z2r,
                condT=condT,
                wc1=wc1T,
                wc2=wc2T,
                wh2=wh2,
                qw=qw,
                vb=vb,
                wcv=wcv,
                wout=woutT,
                biases=bias_pack,
                brep=brep,
                mask=mask,
            )
        )
    return in_maps


def _assemble_output(x, results):
    z = np.empty((B, T, D), np.float32)
    ld = np.empty((B,), np.float32)
    for c in range(NCORES):
        z[c * NB : (c + 1) * NB, :T2] = x[c * NB : (c + 1) * NB, :T2]
        z2n = np.asarray(results[c]["z2n"], np.float32)
        z[c * NB : (c + 1) * NB, T2:] = (
            z2n.reshape(4, T2, NCHUNK, 2).transpose(2, 0, 1, 3).reshape(NB, T2, 2)
        )
        ld[c * NB : (c + 1) * NB] = np.asarray(results[c]["ld"], np.float32).reshape(
            NB
        )
    return z, ld


def run(inputs, trace=False, trace_cores=None):
    """Run on 8 NeuronCores; returns ((z, logdet), BassKernelResults)."""
    from concourse.bass_utils import run_bass_kernel_spmd

    nc = _get_program()
    in_maps = _host_inputs(inputs)
    res = run_bass_kernel_spmd(
        nc,
        in_maps,
        list(range(NCORES)),
        trace=trace,
        trace_cores=trace_cores if trace_cores is not None else list(range(NCORES)),
    )
    x = np.asarray(inputs["x"], np.float32)
    return _assemble_output(x, res.results), res


def kernel(**inputs):
    (z, ld), _ = run(inputs, trace=False)
    return z, ld


if __name__ == "__main__":
    print("build only:", _get_program())


# revision 37
# speedup vs baseline: 1.2424x; 1.0166x over previous
"""Trainium2 Bass kernel for nn_NlsqCond (ConvFlow NLSQ coupling layer).

Strategy: pure data parallel over batch B=256 -> 32 samples per core on 8
NeuronCores. Convs are computed as 3 shifted matmuls accumulating in PSUM
over a gap-column activation layout (34 columns per sample, zero guard
columns), so the k=3/pad=1 conv needs no boundary special-casing. Weights
are transposed host-side into lhsT layout and cast to bf16 (fp32 PSUM
accumulation); measured end-to-end error vs fp32 reference is ~1e-4.
The final projection is computed transposed ([cols, 10]) so the NLSQ
elementwise tail runs with full 128-partition parallelism; the per-sample
logdet partition-reduction is done with a small mask matmul.
"""

import math

import numpy as np

B, T, D, H, COND = 256, 64, 2, 512, 8
T2 = T // 2                      # 32
NCORES = 8
NB = B // NCORES                 # 32 samples per core
ST = T2 + 2                      # 34: per-sample column stride (zero gaps)
WCOLS = NB * ST                  # 1088
NG = 2                           # PSUM column groups per matmul set
GS = NB // NG                    # 16 samples per group
NCHUNK = NB * T2 // 128          # 8 column chunks of 128 for final proj
LOG_A = math.log(8.0 * math.sqrt(3.0) / 9.0 - 0.05)

_CACHE = {}


def _build_program():
    import concourse.bacc as bacc
    import concourse.mybir as mybir
    import concourse.tile as tile

    f32 = mybir.dt.float32
    bf16 = mybir.dt.bfloat16
    AF = mybir.ActivationFunctionType
    Alu = mybir.AluOpType

    nc = bacc.Bacc("TRN2", target_bir_lowering=False, debug=False)

    # ---- DRAM I/O ----
    # z1 in gap layout (zero guard columns), rows replicated for the 3 conv
    # shifts: row (k*2+d) col m = z1_gap[d, m+k]. Feeds conv0's h-half as a
    # single K=6 matmul per window.
    d_z1g = nc.dram_tensor("z1g", [6, WCOLS], bf16, kind="ExternalInput")
    d_z2r = nc.dram_tensor("z2r", [128, NCHUNK, 2], f32, kind="ExternalInput")
    d_condT = nc.dram_tensor("condT", [2 * COND, NB], bf16, kind="ExternalInput")
    d_wc1 = nc.dram_tensor("wc1", [2 * COND, H], bf16, kind="ExternalInput")
    # conv/linear weights packed partition-major so each DMA moves one large
    # contiguous chunk per partition (descriptor-rate, not bandwidth, limits
    # small-row DMAs)
    d_wc2 = nc.dram_tensor("wc2", [128, 4, H], bf16, kind="ExternalInput")
    # conv0 h-half collapsed to rank 6: wh6[k*2+d] = (w_conv0[:, :H, k] @ we).T
    d_wh6 = nc.dram_tensor("wh6", [6, H], bf16, kind="ExternalInput")
    # conv0 cond-half collapsed to per-sample vectors: types (sum_k, k=0, k=2)
    d_qw = nc.dram_tensor("qw", [128, 4, 3, H], bf16, kind="ExternalInput")
    # b_embed fold rank-1 terms per type
    d_vb = nc.dram_tensor("vb", [1, 3, H], bf16, kind="ExternalInput")
    d_wcv = nc.dram_tensor("wcv", [4, 128, 4, 3, H], bf16, kind="ExternalInput")
    d_wout = nc.dram_tensor("wout", [128, 4, 10], bf16, kind="ExternalInput")
    d_bias = nc.dram_tensor("biases", [128, 8, 4], f32, kind="ExternalInput")
    d_brep = nc.dram_tensor("brep", [128, NCHUNK, 10], f32, kind="ExternalInput")
    d_mask = nc.dram_tensor("mask", [128, 4], f32, kind="ExternalInput")
    d_z2n = nc.dram_tensor("z2n", [128, NCHUNK, 2], f32, kind="ExternalOutput")
    d_ld = nc.dram_tensor("ld", [NCHUNK, 4], f32, kind="ExternalOutput")

    with tile.TileContext(nc) as tc:
        with (
            tc.tile_pool(name="w", bufs=1) as wp,
            tc.tile_pool(name="act", bufs=1) as ap_,
            tc.tile_pool(name="ew", bufs=1) as ewp,
            tc.tile_pool(name="ps", bufs=8, space="PSUM") as pp,
        ):
            def mk(pool, shape, dtype, tag):
                return pool.tile(shape, dtype, tag=tag, name=tag)

            # ---- SBUF loads (small inputs first so PE can start early) ----
            z1g_sb = mk(ap_, [6, WCOLS], bf16, "z1g")
            nc.sync.dma_start(z1g_sb[:], d_z1g.ap())
            wh6_sb = mk(wp, [6, H], bf16, "wh6")
            nc.sync.dma_start(wh6_sb[:], d_wh6.ap())
            condT_sb = mk(ap_, [2 * COND, NB], bf16, "condT")
            nc.sync.dma_start(condT_sb[:], d_condT.ap())
            wc1_sb = mk(wp, [2 * COND, H], bf16, "wc1")
            nc.sync.dma_start(wc1_sb[:], d_wc1.ap())
            bias_sb = mk(wp, [128, 8, 4], f32, "bias")
            nc.sync.dma_start(bias_sb[:], d_bias.ap())
            wc2_sb = mk(wp, [128, 4, H], bf16, "wc2")
            nc.sync.dma_start(wc2_sb[:], d_wc2.ap())
            vb_sb = mk(wp, [1, 3, H], bf16, "vb")
            nc.sync.dma_start(vb_sb[:], d_vb.ap())
            qw_sb = mk(wp, [128, 4, 3, H], bf16, "qw")
            nc.sync.dma_start(qw_sb[:], d_qw.ap())
            wcv_sb = [mk(wp, [128, 4, 3, H], bf16, f"wcv_{l}") for l in range(4)]
            for l in range(4):
                nc.sync.dma_start(wcv_sb[l][:], d_wcv.ap()[l])
            wout_sb = mk(wp, [128, 4, 10], bf16, "wout")
            nc.sync.dma_start(wout_sb[:], d_wout.ap())
            brep_sb = mk(wp, [128, NCHUNK, 10], f32, "brep")
            nc.sync.dma_start(brep_sb[:], d_brep.ap())
            mask_sb = mk(wp, [128, 4], f32, "mask")
            nc.sync.dma_start(mask_sb[:], d_mask.ap())
            z2r_sb = mk(ewp, [128, NCHUNK, 2], f32, "z2r")
            nc.sync.dma_start(z2r_sb[:], d_z2r.ap())

            def bias_ap(idx, mt):
                return bias_sb[:, idx, mt : mt + 1]

            # ---- PE warm-up scratch (memset first so the PE can spin ASAP) ----
            wu_l = mk(ap_, [128, 128], bf16, "wu_l")
            wu_r = mk(ap_, [128, 512], bf16, "wu_r")
            nc.vector.memset(wu_l[:], 0.0)
            nc.vector.memset(wu_r[:], 0.0)

            # ---- activation ping-pong buffers (gap layout) ----
            # Only the gap guard columns need zeroing; valid columns are
            # always written before they are read.
            actB = [mk(ap_, [128, WCOLS], bf16, f"B{i}") for i in range(4)]
            actC = [mk(ap_, [128, WCOLS], bf16, f"C{i}") for i in range(4)]
            for ti, t_ in enumerate(actB + actC):
                v = t_[:].rearrange("p (s c) -> p s c", c=ST)
                eng = nc.vector if ti % 2 else nc.gpsimd
                eng.memset(v[:, :, 0:1], 0.0)
                eng.memset(v[:, :, ST - 1 : ST], 0.0)
            ones_sb = mk(ap_, [1, NB], bf16, "ones")
            nc.vector.memset(ones_sb[:], 1.0)

            def valid(tl, g=None):
                v = tl[:].rearrange("p (s c) -> p s c", c=ST)
                if g is None:
                    return v[:, :, 1 : 1 + T2]
                return v[:, g * GS : (g + 1) * GS, 1 : 1 + T2]

            # ---- PE warm-up spin ----
            # The PE HAM clock gate starts at 1.2 GHz and only releases to
            # 2.4 GHz after ~3.4us of sustained activity. Matmul on scratch
            # zeros while the weight DMAs stream in, so the real conv stack
            # runs warm from its first instruction.
            ps_w = mk(pp, [128, 512], f32, "ps")
            for i in range(16):
                nc.tensor.matmul(
                    ps_w[:], wu_l[:], wu_r[:], start=(i == 0), stop=(i == 15)
                )

            # preload the ACT transcendental tables so the elementwise tail
            # doesn't pay the table-swap latency
            scr = mk(ewp, [1, 4], f32, "scr")
            nc.scalar.activation(scr[:, 0:1], wu_l[:1, 0:1], AF.Exp)
            nc.scalar.activation(scr[:, 1:2], wu_l[:1, 0:1], AF.Tanh)
            nc.scalar.activation(scr[:, 2:3], wu_l[:1, 0:1], AF.Ln, bias=1.0)

            # ---- cond MLP: c2 = relu(W2 relu(W1 c + b1) + b2) ----
            c1_sb = [mk(ap_, [128, NB], bf16, f"c1_{i}") for i in range(4)]
            c2_sb = [mk(ap_, [128, NB], bf16, f"c2_{i}") for i in range(4)]
            for mt in range(4):
                ps = mk(pp, [128, 512], f32, "ps")
                nc.tensor.matmul(
                    ps[:, :NB],
                    wc1_sb[:, mt * 128 : (mt + 1) * 128],
                    condT_sb[:],
                    start=True,
                    stop=True,
                )
                nc.scalar.activation(
                    c1_sb[mt][:], ps[:, :NB], AF.Relu, bias=bias_ap(1, mt)
                )
            for mt in range(4):
                ps = mk(pp, [128, 512], f32, "ps")
                for kt in range(4):
                    nc.tensor.matmul(
                        ps[:, :NB],
                        wc2_sb[:, kt, mt * 128 : (mt + 1) * 128],
                        c1_sb[kt][:],
                        start=(kt == 0),
                        stop=(kt == 3),
                    )
                nc.scalar.activation(
                    c2_sb[mt][:], ps[:, :NB], AF.Relu, bias=bias_ap(2, mt)
                )

            # ---- conv0 cond-half collapsed to per-sample vectors ----
            # q[:, 0, s] = (sum_k W0k_c) @ c2[s] + sum_k(W0k_h @ b_embed)
            # q[:, 1, s] = W00_c @ c2[s] + W00_h @ b_embed   (t=0 correction)
            # q[:, 2, s] = W02_c @ c2[s] + W02_h @ b_embed   (t=31 correction)
            q_sb = []
            for mt in range(4):
                ps = mk(pp, [128, 512], f32, "ps")
                for ty in range(3):
                    for kt in range(4):
                        nc.tensor.matmul(
                            ps[:, ty * NB : (ty + 1) * NB],
                            qw_sb[:, kt, ty, mt * 128 : (mt + 1) * 128],
                            c2_sb[kt][:],
                            start=(kt == 0),
                            stop=False,
                        )
                    nc.tensor.matmul(
                        ps[:, ty * NB : (ty + 1) * NB],
                        vb_sb[:, ty, mt * 128 : (mt + 1) * 128],
                        ones_sb[:],
                        start=False,
                        stop=True,
                    )
                # keep the HAM activity monitor fed through this low-K region
                nc.tensor.matmul(ps_w[:], wu_l[:], wu_r[:], start=True, stop=True)
                nc.tensor.matmul(ps_w[:], wu_l[:], wu_r[:], start=True, stop=True)
                q = mk(ewp, [128, 3, NB], f32, f"q_{mt}")
                nc.vector.tensor_copy(
                    q[:], ps[:, : 3 * NB].rearrange("p (y s) -> p y s", s=NB)
                )
                q_sb.append(q)

            # ---- conv stack ----
            # Matmul moving operands must be single-free-dim, so each conv
            # matmul streams a contiguous window of the gap layout; outputs
            # at gap positions are garbage and simply never read back.
            WINDOWS = [(0, 15), (15, 15), (30, 2)]  # (sample base, n samples)
            for L in range(5):
                srcs = [actB, actC, actB, actC][L - 1] if L > 0 else None
                ktn = 4 if L > 0 else 0
                wts = wcv_sb[L - 1][:] if L > 0 else None
                dsts = actB if L % 2 == 0 else actC
                for mt in range(4):
                    pss = [mk(pp, [128, 512], f32, "ps") for _ in WINDOWS]
                    if L == 0:
                        # h-half: single K=6 matmul per window (the 3 conv
                        # shifts are baked into the replicated z1g rows)
                        for wi, (sb, ns) in enumerate(WINDOWS):
                            n = ns * ST - 2
                            base = sb * ST
                            nc.tensor.matmul(
                                pss[wi][:, :n],
                                wh6_sb[:, mt * 128 : (mt + 1) * 128],
                                z1g_sb[:, base : base + n],
                                start=True,
                                stop=True,
                            )
                    nacc = 3 * ktn
                    i = 0
                    # kt outer: matches weight-DMA arrival order
                    for kt in range(ktn):
                        for k in range(3):
                            lhsT = wts[:, kt, k, mt * 128 : (mt + 1) * 128]
                            for wi, (sb, ns) in enumerate(WINDOWS):
                                n = ns * ST - 2
                                base = sb * ST + k
                                nc.tensor.matmul(
                                    pss[wi][:, :n],
                                    lhsT,
                                    srcs[kt][:, base : base + n],
                                    start=(i == 0),
                                    stop=(i == nacc - 1),
                                )
                            i += 1
                    for wi, (sb, ns) in enumerate(WINDOWS):
                        pv3 = pss[wi][:, : ns * ST].rearrange(
                            "p (s c) -> p s c", c=ST
                        )
                        if L == 0:
                            # add broadcast cond term + boundary corrections
                            qv = q_sb[mt][:, 0, sb : sb + ns]
                            nc.vector.tensor_add(
                                pv3[:, :, 0:T2],
                                pv3[:, :, 0:T2],
                                qv.unsqueeze(2).broadcast_to([128, ns, T2]),
                            )
                            nc.vector.tensor_sub(
                                pv3[:, :, 0:1],
                                pv3[:, :, 0:1],
                                q_sb[mt][:, 1, sb : sb + ns].unsqueeze(2),
                            )
                            nc.vector.tensor_sub(
                                pv3[:, :, T2 - 1 : T2],
                                pv3[:, :, T2 - 1 : T2],
                                q_sb[mt][:, 2, sb : sb + ns].unsqueeze(2),
                            )
                        pv_ = pv3[:, :, 0:T2]
                        dv_ = dsts[mt][:].rearrange("p (s c) -> p s c", c=ST)[
                            :, sb : sb + ns, 1 : 1 + T2
                        ]
                        # split the PSUM->SBUF relu copies across ACT and DVE
                        if (mt * 3 + wi) % 2 == 0:
                            nc.scalar.activation(
                                dv_, pv_, AF.Relu, bias=bias_ap(3 + L, mt)
                            )
                        else:
                            nc.vector.tensor_scalar(
                                dv_,
                                pv_,
                                bias_ap(3 + L, mt),
                                0.0,
                                Alu.add,
                                Alu.max,
                            )
            srcs = actB

            # ---- compact the final activation (drop gap columns) ----
            h5 = [mk(ap_, [128, NB * T2], bf16, f"h5_{i}") for i in range(4)]
            for kt in range(4):
                eng = nc.scalar if kt % 2 == 0 else nc.vector
                if kt % 2 == 0:
                    nc.scalar.copy(
                        h5[kt][:].rearrange("p (s t) -> p s t", t=T2),
                        valid(srcs[kt]),
                    )
                else:
                    nc.vector.tensor_copy(
                        h5[kt][:].rearrange("p (s t) -> p s t", t=T2),
                        valid(srcs[kt]),
                    )

            # ---- final projection (transposed): out[col, 10] ----
            pso = mk(pp, [128, 512], f32, "ps")
            for j in range(NCHUNK):
                for kt in range(4):
                    nc.tensor.matmul(
                        pso[:, j * 10 : (j + 1) * 10],
                        h5[kt][:, j * 128 : (j + 1) * 128],
                        wout_sb[:, kt],
                        start=(kt == 0),
                        stop=(kt == 3),
                    )

            # ---- NLSQ elementwise tail ----
            def ew(tag):
                return mk(ewp, [128, NCHUNK, 2], f32, tag)

            params = mk(ewp, [128, NCHUNK, 10], f32, "params")
            nc.vector.tensor_add(
                params[:],
                pso[:, : NCHUNK * 10].rearrange("p (j q) -> p j q", q=10),
                brep_sb[:],
            )
            pv = params[:].rearrange("p j (a q) -> p j a q", q=5)
            P0, P1, P2, P3, P4 = (pv[:, :, :, i] for i in range(5))

            loga_sb = mk(ewp, [128, 1], f32, "loga")
            nc.vector.memset(loga_sb[:], LOG_A)

            # group ACT functions (Exp x3, then Tanh, Ln last) to minimize
            # activation-table reloads
            u = ew("u")
            nc.vector.tensor_tensor(u[:], P1, P3, op=Alu.subtract)
            b_ = ew("b_")
            nc.scalar.activation(b_[:], P1, AF.Exp, scale=0.4)
            d_ = ew("d_")
            nc.scalar.activation(d_[:], P3, AF.Exp, scale=0.4)
            e = ew("e")
            nc.scalar.activation(e[:], u[:], AF.Exp, scale=0.4, bias=loga_sb[:])
            th = ew("th")
            nc.scalar.activation(th[:], P2, AF.Tanh, scale=0.3)
            c_ = ew("c_")
            nc.vector.tensor_mul(c_[:], th[:], e[:])
            t1 = ew("t1")
            nc.vector.tensor_mul(t1[:], d_[:], z2r_sb[:])
            arg = ew("arg")
            nc.vector.tensor_add(arg[:], t1[:], P4)
            sq = ew("sq")
            nc.vector.tensor_mul(sq[:], arg[:], arg[:])
            den = ew("den")
            nc.vector.tensor_scalar_add(den[:], sq[:], 1.0)
            rcp = ew("rcp")
            nc.vector.reciprocal(rcp[:], den[:])
            t2 = ew("t2")
            nc.vector.tensor_mul(t2[:], b_[:], z2r_sb[:])
            t3 = ew("t3")
            nc.vector.tensor_mul(t3[:], c_[:], rcp[:])
            s1 = ew("s1")
            nc.vector.tensor_add(s1[:], P0, t2[:])
            z2n_sb = ew("z2n_sb")
            nc.vector.tensor_add(z2n_sb[:], s1[:], t3[:])
            nc.sync.dma_start(d_z2n.ap(), z2n_sb[:])

            t4 = ew("t4")
            nc.vector.tensor_mul(t4[:], c_[:], d_[:])
            t5 = ew("t5")
            nc.vector.tensor_mul(t5[:], t4[:], arg[:])
            t6 = ew("t6")
            nc.vector.tensor_mul(t6[:], t5[:], rcp[:])
            t7 = ew("t7")
            nc.vector.tensor_mul(t7[:], t6[:], rcp[:])
            inner = ew("inner")
            nc.vector.scalar_tensor_tensor(
                inner[:], t7[:], -2.0, b_[:], op0=Alu.mult, op1=Alu.add
            )
            lg = ew("lg")
            nc.scalar.activation(lg[:], inner[:], AF.Ln)

            lg2 = mk(ewp, [128, NCHUNK], f32, "lg2")
            nc.vector.tensor_add(lg2[:], lg[:, :, 0], lg[:, :, 1])
            psl = mk(pp, [128, 512], f32, "ps")
            nc.tensor.matmul(
                psl[:NCHUNK, :4], lg2[:], mask_sb[:], start=True, stop=True
            )
            ld_sb = mk(ewp, [NCHUNK, 4], f32, "ld_sb")
            nc.vector.tensor_copy(ld_sb[:], psl[:NCHUNK, :4])
            nc.sync.dma_start(d_ld.ap(), ld_sb[:])

    nc.compile()
    return nc


def _get_program():
    if "nc" not in _CACHE:
        _CACHE["nc"] = _build_program()
    return _CACHE["nc"]


def _host_inputs(inputs):
    import ml_dtypes

    bf16 = ml_dtypes.bfloat16
    f32 = np.float32

    x = np.asarray(inputs["x"], f32)
    cond = np.asarray(inputs["cond"], f32)

    wc1T = np.ascontiguousarray(inputs["w_c1"].T).astype(bf16)
    # partition-major packing: [128, ...] with large contiguous per-partition
    # chunks so SBUF DMAs are few big descriptors per partition
    wc2T = np.ascontiguousarray(
        inputs["w_c2"].T.reshape(4, 128, H).transpose(1, 0, 2)
    ).astype(bf16)
    # conv0 h-half collapsed through the rank-2 embedding; rows (k*2+d)
    w0 = np.asarray(inputs["w_conv0"], f32)
    we2 = np.asarray(inputs["w_embed"], f32)[:, :2]
    wh6 = np.ascontiguousarray(
        np.einsum("ock,cd->kdo", w0[:, :H, :], we2).reshape(6, H)
    ).astype(bf16)
    # conv0 cond-half collapsed to per-sample vectors (types: sum_k, k0, k2)
    S = w0[:, H:, :]
    mats = np.stack([(S[:, :, 0] + S[:, :, 1] + S[:, :, 2]).T, S[:, :, 0].T, S[:, :, 2].T])
    qw = np.ascontiguousarray(
        mats.reshape(3, 4, 128, H).transpose(2, 1, 0, 3)
    ).astype(bf16)
    # b_embed folded through conv0 (rank-1 terms per type)
    be = np.asarray(inputs["b_embed"], f32)
    v = np.stack([w0[:, :H, k] @ be for k in range(3)])  # [3, H(out)]
    vb = np.ascontiguousarray(
        np.stack([v[0] + v[1] + v[2], v[0], v[2]])[None, :, :]
    ).astype(bf16)
    wcv = np.stack(
        [
            np.ascontiguousarray(
                np.transpose(inputs[f"w_conv{i}"], (1, 2, 0))
                .reshape(4, 128, 3, H)
                .transpose(1, 0, 2, 3)
            )
            for i in (1, 2, 3, 4)
        ]
    ).astype(bf16)
    woutT = np.ascontiguousarray(
        inputs["w_out"].T.reshape(4, 128, 10).transpose(1, 0, 2)
    ).astype(bf16)
    bias_all = np.stack(
        [inputs["b_embed"], inputs["b_c1"], inputs["b_c2"]]
        + [inputs[f"b_conv{i}"] for i in range(5)]
    ).astype(f32)
    bias_pack = np.ascontiguousarray(
        bias_all.reshape(8, 4, 128).transpose(2, 0, 1)
    ).astype(f32)
    brep = np.ascontiguousarray(
        np.broadcast_to(inputs["b_out"].astype(f32), (128, NCHUNK, 10))
    )
    mask = np.zeros((128, 4), f32)
    mask[np.arange(128), np.arange(128) // 32] = 1.0

    in_maps = []
    for c in range(NCORES):
        xs = x[c * NB : (c + 1) * NB]
        z1 = xs[:, :T2]
        z2 = xs[:, T2:]
        z1p = np.zeros((2, WCOLS + 2), np.float32)
        z1p[:, :WCOLS].reshape(2, NB, ST)[:, :, 1 : 1 + T2] = z1.transpose(2, 0, 1)
        z1g = np.stack(
            [z1p[dd, k : k + WCOLS] for k in range(3) for dd in range(2)]
        ).astype(bf16)
        z2r = np.ascontiguousarray(
            z2.reshape(NCHUNK, 4, T2, 2).transpose(1, 2, 0, 3)
        ).reshape(128, NCHUNK, 2)
        condT = np.ascontiguousarray(
            cond[c * NB : (c + 1) * NB].reshape(NB, 2 * COND).T
        ).astype(bf16)
        in_maps.append(
            dict(
                z1g=z1g,
                z2r=z2r,
                condT=condT,
                wc1=wc1T,
                wc2=wc2T,
                wh6=wh6,
                qw=qw,
                vb=vb,
                wcv=wcv,
                wout=woutT,
                biases=bias_pack,
                brep=brep,
                mask=mask,
            )
        )
    return in_maps


def _assemble_output(x, results):
    z = np.empty((B, T, D), np.float32)
    ld = np.empty((B,), np.float32)
    for c in range(NCORES):
        z[c * NB : (c + 1) * NB, :T2] = x[c * NB : (c + 1) * NB, :T2]
        z2n = np.asarray(results[c]["z2n"], np.float32)
        z[c * NB : (c + 1) * NB, T2:] = (
            z2n.reshape(4, T2, NCHUNK, 2).transpose(2, 0, 1, 3).reshape(NB, T2, 2)
        )
        ld[c * NB : (c + 1) * NB] = np.asarray(results[c]["ld"], np.float32).reshape(
            NB
        )
    return z, ld


def run(inputs, trace=False, trace_cores=None):
    """Run on 8 NeuronCores; returns ((z, logdet), BassKernelResults)."""
    from concourse.bass_utils import run_bass_kernel_spmd

    nc = _get_program()
    in_maps = _host_inputs(inputs)
    res = run_bass_kernel_spmd(
        nc,
        in_maps,
        list(range(NCORES)),
        trace=trace,
        trace_cores=trace_cores if trace_cores is not None else list(range(NCORES)),
    )
    x = np.asarray(inputs["x"], np.float32)
    return _assemble_output(x, res.results), res


def kernel(**inputs):
    (z, ld), _ = run(inputs, trace=False)
    return z, ld


if __name__ == "__main__":
    print("build only:", _get_program())
